# revision 1
# baseline (speedup 1.0000x reference)
"""CrossFeatureFusion TRN2 kernel.

out[i] = x[i] + sum_{j != i} (x[j] @ W[i,j]^T + b[i,j])
x: [4, 65536, 256] f32, W: [4, 4, 256, 256] f32, b: [4, 4, 256] f32.

Strategy (data-parallel over N, 8 NeuronCores, no collectives):
  - Host: transpose x to feature-major shards xt[core][j, fc, k, n] so the
    contraction dim (f = fc*128 + k) lies on SBUF partitions with no
    on-chip transpose.
  - Host: build block weights M[i][j] = (I if j == i else W[i,j]^T), packed
    per output pair (0,1) / (2,3) as the moving operand.  The identity
    diagonal folds the residual "+ x[i]" into the PSUM accumulation.
  - Device: per 128-row block, 16 fp32r matmuls of [K=128] x [N=512]
    accumulate the full fused output for all 4 modalities in 2 PSUM banks;
    DVE adds the precomputed bias sums while draining PSUM -> SBUF;
    HWDGE DMAs move x-shards in and outputs back.
  - fp32r (TF32-like PE mode) runs at ~1 row/cycle for moving dim >= 256;
    measured rel err vs fp32 reference ~1.5e-4.
"""

import sys

if "/opt/trn_rl_repo" not in sys.path:
    sys.path.insert(0, "/opt/trn_rl_repo")

import numpy as np

M, N, D = 4, 65536, 256
N_CORES = 8
NSH = N // N_CORES  # rows per core
NBLK = NSH // 128  # 128-row blocks per core
PAIRS = ((0, 1), (2, 3))

_CACHE = {}


def _build_nc(nsh=NSH, repeat=1, xbufs=4, obufs=4, pbufs=4):
    from concourse import bacc
    import concourse.mybir as mybir
    import concourse.tile as tile

    f32 = mybir.dt.float32
    f32r = mybir.dt.float32r
    nblk = nsh // 128

    nc = bacc.Bacc(debug=False)
    xt_d = nc.dram_tensor("xt", [M, 2, 128, nsh], f32r, kind="ExternalInput")
    wp_d = nc.dram_tensor("wp", [2, 8, 128, 512], f32r, kind="ExternalInput")
    bb_d = nc.dram_tensor("bb", [1, 2, 512], f32, kind="ExternalInput")
    out_d = nc.dram_tensor("out", [M, nsh, D], f32, kind="ExternalOutput")

    with tile.TileContext(nc) as tc:
        with (
            tc.tile_pool(name="wsb", bufs=1) as wpool,
            tc.tile_pool(name="xt", bufs=xbufs) as xpool,
            tc.tile_pool(name="osb", bufs=obufs) as opool,
            tc.tile_pool(name="psum", bufs=pbufs, space="PSUM") as ppool,
        ):
            w_sb = wpool.tile([128, 2, 8, 512], f32r)
            nc.sync.dma_start(out=w_sb[:], in_=wp_d.rearrange("p c k e -> k p c e"))
            bias_sb = wpool.tile([128, 2, 512], f32)
            nc.sync.dma_start(
                out=bias_sb[:], in_=bb_d[:].to_broadcast([128, 2, 512])
            )

            def body():
                for nb in range(nblk):
                    n0 = nb * 128
                    xt_sb = xpool.tile([128, M, 2, 128], f32r, name="xt_sb", tag="xt")
                    nc.sync.dma_start(
                        out=xt_sb[:],
                        in_=xt_d[:, :, :, n0 : n0 + 128].rearrange(
                            "j f k n -> k j f n"
                        ),
                    )
                    pss = [
                        ppool.tile([128, 512], f32, tag=f"ps{p}", name=f"ps{p}_{nb}")
                        for p in range(2)
                    ]
                    for c in range(8):
                        j, fc = c >> 1, c & 1
                        for p in range(2):
                            nc.tensor.matmul(
                                pss[p][:],
                                lhsT=xt_sb[:, j, fc, :],
                                rhs=w_sb[:, p, c, :],
                                start=(c == 0),
                                stop=(c == 7),
                            )
                    for p in range(2):
                        o_sb = opool.tile(
                            [128, 2, 256], f32, name=f"osb{p}_{nb}", tag="osb"
                        )
                        nc.vector.tensor_add(
                            out=o_sb[:].rearrange("n i e -> n (i e)"),
                            in0=pss[p][:],
                            in1=bias_sb[:, p, :],
                        )
                        nc.sync.dma_start(
                            out=out_d[2 * p : 2 * p + 2, n0 : n0 + 128, :].rearrange(
                                "i n e -> n i e"
                            ),
                            in_=o_sb[:],
                        )

            if repeat > 1:
                with tc.For_i(0, repeat, 1):
                    body()
            else:
                body()
    nc.finalize()
    return nc


def _build_nc_v2(nsh=NSH, repeat=1, xbufs=3, obufs=6, pbufs=2):
    """out^T formulation: W stationary, xt moving -> PSUM holds out^T[i]
    chunks [128 e, 512 n].  No identity matmuls: the residual "+x[i]" is a
    direct DVE add from the (already transposed) xt tile, fused with the
    bias add in one scalar_tensor_tensor while draining PSUM.  Host
    un-transposes the [4, 2, 128, nsh] output during gather."""
    from concourse import bacc
    import concourse.mybir as mybir
    import concourse.tile as tile

    f32 = mybir.dt.float32
    f32r = mybir.dt.float32r
    NB = 512  # rows per block
    nblk = nsh // NB
    add = mybir.AluOpType.add

    nc = bacc.Bacc(debug=False)
    xt_d = nc.dram_tensor("xt", [M, 2, 128, nsh], f32r, kind="ExternalInput")
    wst_d = nc.dram_tensor("wst", [8, 6, 128, 128], f32r, kind="ExternalInput")
    bbt_d = nc.dram_tensor("bbt", [8, 128], f32, kind="ExternalInput")
    out_d = nc.dram_tensor("out", [M, 2, 128, nsh], f32, kind="ExternalOutput")

    jl = [[j for j in range(M) if j != i] for i in range(M)]

    with tile.TileContext(nc) as tc:
        with (
            tc.tile_pool(name="wsb", bufs=1) as wpool,
            tc.tile_pool(name="xt", bufs=xbufs) as xpool,
            tc.tile_pool(name="osb", bufs=obufs) as opool,
            tc.tile_pool(name="psum", bufs=pbufs, space="PSUM") as ppool,
        ):
            w_sb = wpool.tile([128, 8, 6, 128], f32r)
            nc.sync.dma_start(out=w_sb[:], in_=wst_d.rearrange("t c k m -> k t c m"))
            bias_sb = wpool.tile([128, 8], f32)
            nc.sync.dma_start(out=bias_sb[:], in_=bbt_d.rearrange("t k -> k t"))

            def body():
                for nb in range(nblk):
                    n0 = nb * NB
                    xt_sb = xpool.tile([128, M, 2, NB], f32r, name="xt_sb", tag="xt")
                    nc.sync.dma_start(
                        out=xt_sb[:],
                        in_=xt_d[:, :, :, n0 : n0 + NB].rearrange(
                            "j f k n -> k j f n"
                        ),
                    )
                    for half in range(2):
                        pss = [
                            ppool.tile(
                                [128, NB], f32, tag=f"ps{t}", name=f"ps{t}_{nb}"
                            )
                            for t in range(4)
                        ]
                        for tt in range(4):
                            tg = half * 4 + tt
                            i = tg >> 1
                            for cc in range(6):
                                fc = cc & 1
                                j = jl[i][cc >> 1]
                                nc.tensor.matmul(
                                    pss[tt][:],
                                    lhsT=w_sb[:, tg, cc, :],
                                    rhs=xt_sb[:, j, fc, :],
                                    start=(cc == 0),
                                    stop=(cc == 5),
                                )
                        for tt in range(4):
                            tg = half * 4 + tt
                            i, ec = tg >> 1, tg & 1
                            o_sb = opool.tile(
                                [128, NB], f32, name=f"osb{tg}_{nb}", tag="osb"
                            )
                            nc.vector.scalar_tensor_tensor(
                                out=o_sb[:],
                                in0=pss[tt][:],
                                scalar=bias_sb[:, tg : tg + 1],
                                in1=xt_sb[:, i, ec, :].bitcast(f32),
                                op0=add,
                                op1=add,
                            )
                            nc.sync.dma_start(
                                out=out_d[i, ec, :, n0 : n0 + NB], in_=o_sb[:]
                            )

            if repeat > 1:
                with tc.For_i(0, repeat, 1):
                    body()
            else:
                body()
    nc.finalize()
    return nc


def _get_exec(**build_kwargs):
    """Build (once per config) the jitted 8-core executor. Returns a callable
    run(xt_g, wst_g, bbt_g, n_iters) -> out_g with global concat arrays."""
    key = tuple(sorted(build_kwargs.items()))
    if key in _CACHE:
        return _CACHE[key]

    import jax
    import jax.numpy as jnp
    from jax.sharding import Mesh, PartitionSpec
    from jax.experimental.shard_map import shard_map
    from concourse import bass2jax

    nc = _build_nc_v2(**build_kwargs)
    bass2jax.install_neuronx_cc_hook()

    in_names = ["xt", "wst", "bbt", "out"]
    if nc.partition_id_tensor is not None:
        in_names.append(nc.partition_id_tensor.name)
    out_names = ["out"]
    out_aval = jax.core.ShapedArray((M, 2, 128, NSH), np.float32)

    def _body(xt, wst, bbt, out_zero):
        operands = [xt, wst, bbt, out_zero]
        if nc.partition_id_tensor is not None:
            operands.append(bass2jax.partition_id_tensor())
        outs = bass2jax._bass_exec_p.bind(
            *operands,
            out_avals=(out_aval,),
            in_names=tuple(in_names),
            out_names=tuple(out_names),
            lowering_input_output_aliases=(),
            sim_require_finite=True,
            sim_require_nnan=True,
            nc=nc,
        )
        return tuple(outs)

    devices = jax.devices()[:N_CORES]
    mesh = Mesh(np.asarray(devices), ("core",))
    sharded = jax.jit(
        shard_map(
            _body,
            mesh=mesh,
            in_specs=(PartitionSpec("core"),) * 4,
            out_specs=(PartitionSpec("core"),),
            check_rep=False,
        ),
        donate_argnums=(3,),
        keep_unused=True,
    )

    sharding = jax.sharding.NamedSharding(mesh, PartitionSpec("core"))
    zeros_fn = jax.jit(
        lambda: jnp.zeros((N_CORES * M, 2, 128, NSH), np.float32),
        out_shardings=sharding,
    )

    def run(xt_g, wst_g, bbt_g, n_iters=1):
        xt_j = jax.device_put(xt_g, sharding)
        wst_j = jax.device_put(wst_g, sharding)
        bbt_j = jax.device_put(bbt_g, sharding)
        outs = None
        for _ in range(n_iters):
            outs = sharded(xt_j, wst_j, bbt_j, zeros_fn())
        jax.block_until_ready(outs)
        return outs[0]

    _CACHE[key] = run
    return run


def _prep_inputs(x, W, b):
    """Host-side shard + layout prep. Returns global concatenated arrays."""
    x = np.asarray(x, dtype=np.float32)
    W = np.asarray(W, dtype=np.float32)
    b = np.asarray(b, dtype=np.float32)
    n = x.shape[1]
    nsh = n // N_CORES

    # xt_g[(c*M + j), fc, k, n] = x[j, c*nsh + n, fc*128 + k]
    x4 = x.reshape(M, N_CORES, nsh, D)
    xt_g = np.ascontiguousarray(x4.transpose(1, 0, 3, 2)).reshape(
        N_CORES * M, 2, 128, nsh
    )

    # Stationary W chunks: wst[(i*2+ec), cc, k, m] = W[i, jl[cc>>1]].T block
    wst = np.empty((8, 6, 128, 128), dtype=np.float32)
    for i in range(M):
        jli = [j for j in range(M) if j != i]
        for ec in range(2):
            t = i * 2 + ec
            for cc in range(6):
                j = jli[cc >> 1]
                fc = cc & 1
                wst[t, cc] = W[i, j][
                    ec * 128 : (ec + 1) * 128, fc * 128 : (fc + 1) * 128
                ].T
    wst_g = np.ascontiguousarray(
        np.broadcast_to(wst[None], (N_CORES, 8, 6, 128, 128))
    ).reshape(N_CORES * 8, 6, 128, 128)

    # bias sums: BS[i] = sum_{j != i} b[i, j];  bbt[(i*2+ec), k]
    bs = b.sum(axis=1) - b[np.arange(M), np.arange(M)]  # [4, 256]
    bbt = bs.reshape(8, 128)
    bbt_g = np.ascontiguousarray(
        np.broadcast_to(bbt[None], (N_CORES, 8, 128))
    ).reshape(N_CORES * 8, 128)

    return xt_g, wst_g, bbt_g


def kernel(x, W, b):
    xt_g, wst_g, bbt_g = _prep_inputs(x, W, b)
    run = _get_exec()
    out_g = run(xt_g, wst_g, bbt_g)
    # out_g: [NC*M, 2, 128, NSH]; out[j, c*NSH+n, ec*128+m] = out_g[c*4+j, ec, m, n]
    out = np.asarray(out_g).reshape(N_CORES, M, 2, 128, NSH)
    out = np.ascontiguousarray(out.transpose(1, 0, 4, 2, 3)).reshape(M, N, D)
    return out



# revision 2
# speedup vs baseline: 21.9448x; 21.9448x over previous
"""CrossFeatureFusion TRN2 kernel, v3 — dense-DMA bf16 formulation.

out[i] = x[i] + sum_{j != i} (x[j] @ W[i,j]^T + b[i,j])
x: [4, 65536, 256] f32, W: [4, 4, 256, 256] f32, b: [4, 4, 256] f32.

Data-parallel over N on 8 cores, no collectives.  Per core (NSH=8192 rows):
  - Host packs x-shards feature-major into per-block dense tiles
    xt[nb, k, (j,fc), n] (bf16) so every device DMA is a contiguous
    [128 x 8KB] transfer — no strided descriptors.
  - W blocks pre-transposed host-side into the stationary operand layout
    wst[k, tg, cc, m] (bf16, 1.5MB, loaded once).
  - Per 512-row block: 48 matmuls (12 (i,j) pairs x 2 ec x 2 fc chunks)
    accumulate out^T[i] in 8 PSUM banks; DVE scalar_tensor_tensor drains
    PSUM + bias + residual x[i] -> bf16 out tile; dense DMA out.
  - Host unpacks/upcasts the bf16 out tiles to the final f32 [4,N,256].
"""

import sys

if "/opt/trn_rl_repo" not in sys.path:
    sys.path.insert(0, "/opt/trn_rl_repo")

import numpy as np
import ml_dtypes

BF16 = ml_dtypes.bfloat16

M, N, D = 4, 65536, 256
N_CORES = 8
NSH = N // N_CORES  # 8192 rows per core
NB = 512  # rows per block
NBLK = NSH // NB  # 16 blocks per core

_CACHE = {}

INPUT_NAMES = ("xt", "wst", "bbt")


def _build_nc(
    nsh=NSH,
    repeat=1,
    xbufs=4,
    obufs=4,
    pbufs=2,
    out_bf16=True,
    stagger=False,
    hints=False,
    unroll=1,
):
    from concourse import bacc
    import concourse.mybir as mybir
    import concourse.tile as tile

    f32 = mybir.dt.float32
    bf16 = mybir.dt.bfloat16
    nblk = nsh // NB
    add = mybir.AluOpType.add
    odt = bf16 if out_bf16 else f32

    nc = bacc.Bacc(debug=False)
    xt_d = nc.dram_tensor("xt", [nblk, 128, 8 * NB], bf16, kind="ExternalInput")
    wst_d = nc.dram_tensor("wst", [128, 8, 6, 128], bf16, kind="ExternalInput")
    bbt_d = nc.dram_tensor("bbt", [128, 8], f32, kind="ExternalInput")
    out_d = nc.dram_tensor("out", [nblk * 2, 128, 4 * NB], odt, kind="ExternalOutput")

    jl = [[j for j in range(M) if j != i] for i in range(M)]

    with tile.TileContext(nc) as tc:
        with (
            tc.tile_pool(name="wsb", bufs=1) as wpool,
            tc.tile_pool(name="xt", bufs=xbufs) as xpool,
            tc.tile_pool(name="osb", bufs=obufs) as opool,
            tc.tile_pool(name="psum", bufs=pbufs, space="PSUM") as ppool,
        ):
            w_sb = wpool.tile([128, 8, 6, 128], bf16)
            nc.sync.dma_start(out=w_sb[:], in_=wst_d[:])
            bias_sb = wpool.tile([128, 8], f32)
            nc.sync.dma_start(out=bias_sb[:], in_=bbt_d[:])

            def body():
                for nb in range(nblk):
                    xt_sb = xpool.tile([128, 8 * NB], bf16, name="xt_sb", tag="xt")
                    nc.sync.dma_start(out=xt_sb[:], in_=xt_d[nb])
                    for half in range(2):
                        pss = [
                            ppool.tile(
                                [128, NB], f32, tag=f"ps{t}", name=f"ps{t}_{nb}"
                            )
                            for t in range(4)
                        ]
                        for tt in range(4):
                            tg = half * 4 + tt
                            i = tg >> 1
                            for cc in range(6):
                                fc = cc & 1
                                j = jl[i][cc >> 1]
                                c8 = j * 2 + fc
                                nc.tensor.matmul(
                                    pss[tt][:],
                                    lhsT=w_sb[:, tg, cc, :],
                                    rhs=xt_sb[:, c8 * NB : (c8 + 1) * NB],
                                    start=(cc == 0),
                                    stop=(cc == 5),
                                )
                        o_sb = opool.tile(
                            [128, 4 * NB], odt, name=f"osb_{nb}_{half}", tag="osb"
                        )
                        for tt in range(4):
                            tg = half * 4 + tt
                            i, ec = tg >> 1, tg & 1
                            r8 = i * 2 + ec
                            nc.vector.scalar_tensor_tensor(
                                out=o_sb[:, tt * NB : (tt + 1) * NB],
                                in0=pss[tt][:],
                                scalar=bias_sb[:, tg : tg + 1],
                                in1=xt_sb[:, r8 * NB : (r8 + 1) * NB],
                                op0=add,
                                op1=add,
                            )
                        nc.sync.dma_start(out=out_d[2 * nb + half], in_=o_sb[:])

            if repeat <= unroll:
                for _ in range(repeat):
                    body()
            else:
                assert repeat % unroll == 0
                kw = {}
                if stagger:
                    kw["staggered_reset"] = True
                if hints:
                    kw["hint_engines"] = (
                        mybir.EngineType.PE,
                        mybir.EngineType.DVE,
                        mybir.EngineType.SP,
                    )
                with tc.For_i(0, repeat // unroll, 1, **kw):
                    for _ in range(unroll):
                        body()
    nc.finalize()
    return nc


def _prep_inputs(x, W, b):
    """Host-side pack. Returns global concatenated (xt_g, wst_g, bbt_g)."""
    x = np.asarray(x, dtype=np.float32)
    W = np.asarray(W, dtype=np.float32)
    b = np.asarray(b, dtype=np.float32)

    # xt_g[(c*NBLK + nb), k, (j*2+fc)*NB + n] = x[j, c*NSH + nb*NB + n, fc*128 + k]
    x6 = x.reshape(M, N_CORES, NBLK, NB, 2, 128)  # j c nb n fc k
    xt_g = np.ascontiguousarray(
        x6.transpose(1, 2, 5, 0, 4, 3)
    ).astype(BF16).reshape(N_CORES * NBLK, 128, 8 * NB)

    # wst[k, tg=(i,ec), cc=(jj,fc), m] = W[i, j][ec*128+m, fc*128+k]
    wst = np.empty((128, 8, 6, 128), dtype=np.float32)
    for i in range(M):
        jli = [j for j in range(M) if j != i]
        for ec in range(2):
            t = i * 2 + ec
            for cc in range(6):
                j = jli[cc >> 1]
                fc = cc & 1
                blk = W[i, j][ec * 128 : (ec + 1) * 128, fc * 128 : (fc + 1) * 128]
                wst[:, t, cc, :] = blk.T
    wst_g = np.ascontiguousarray(
        np.broadcast_to(wst.astype(BF16)[None], (N_CORES, 128, 8, 6, 128))
    ).reshape(N_CORES * 128, 8, 6, 128)

    # bias sums: BS[i] = sum_{j != i} b[i, j]; bbt[k, i*2+ec] = BS[i, ec*128+k]
    bs = b.sum(axis=1) - b[np.arange(M), np.arange(M)]  # [4, 256]
    bbt = np.ascontiguousarray(bs.reshape(M, 2, 128).transpose(2, 0, 1)).reshape(
        128, 8
    )
    bbt_g = np.ascontiguousarray(
        np.broadcast_to(bbt[None], (N_CORES, 128, 8))
    ).reshape(N_CORES * 128, 8)

    return xt_g, wst_g, bbt_g


def _unpack_out(out_g):
    """out_g: [NC*NBLK*2, 128, 4*NB] (bf16) -> out [M, N, D] f32."""
    v = np.asarray(out_g).reshape(N_CORES, NBLK, 2, 128, 2, 2, NB)
    # dims: c nb half m tthi ec n ; i = half*2 + tthi, e = ec*128 + m
    out = v.transpose(2, 4, 0, 1, 6, 5, 3).reshape(M, N, D)
    return np.ascontiguousarray(out).astype(np.float32)


def _get_exec(**build_kwargs):
    """Build (once per config) the jitted 8-core executor."""
    key = tuple(sorted(build_kwargs.items()))
    if key in _CACHE:
        return _CACHE[key]

    import jax
    import jax.numpy as jnp
    from jax.sharding import Mesh, PartitionSpec
    from jax.experimental.shard_map import shard_map
    from concourse import bass2jax

    nc = _build_nc(**build_kwargs)
    bass2jax.install_neuronx_cc_hook()

    out_bf16 = build_kwargs.get("out_bf16", True)
    odt = jnp.bfloat16 if out_bf16 else np.float32

    in_names = list(INPUT_NAMES) + ["out"]
    if nc.partition_id_tensor is not None:
        in_names.append(nc.partition_id_tensor.name)
    out_names = ["out"]
    out_aval = jax.core.ShapedArray((NBLK * 2, 128, 4 * NB), odt)

    def _body(xt, wst, bbt, out_zero):
        operands = [xt, wst, bbt, out_zero]
        if nc.partition_id_tensor is not None:
            operands.append(bass2jax.partition_id_tensor())
        outs = bass2jax._bass_exec_p.bind(
            *operands,
            out_avals=(out_aval,),
            in_names=tuple(in_names),
            out_names=tuple(out_names),
            lowering_input_output_aliases=(),
            sim_require_finite=True,
            sim_require_nnan=True,
            nc=nc,
        )
        return tuple(outs)

    devices = jax.devices()[:N_CORES]
    mesh = Mesh(np.asarray(devices), ("core",))
    sharded = jax.jit(
        shard_map(
            _body,
            mesh=mesh,
            in_specs=(PartitionSpec("core"),) * 4,
            out_specs=(PartitionSpec("core"),),
            check_rep=False,
        ),
        donate_argnums=(3,),
        keep_unused=True,
    )

    sharding = jax.sharding.NamedSharding(mesh, PartitionSpec("core"))
    zeros_fn = jax.jit(
        lambda: jnp.zeros((N_CORES * NBLK * 2, 128, 4 * NB), odt),
        out_shardings=sharding,
    )

    def run(xt_g, wst_g, bbt_g, n_iters=1):
        xt_j = jax.device_put(xt_g, sharding)
        wst_j = jax.device_put(wst_g, sharding)
        bbt_j = jax.device_put(bbt_g, sharding)
        outs = None
        for _ in range(n_iters):
            outs = sharded(xt_j, wst_j, bbt_j, zeros_fn())
        jax.block_until_ready(outs)
        return outs[0]

    _CACHE[key] = run
    return run


def kernel(x, W, b):
    xt_g, wst_g, bbt_g = _prep_inputs(x, W, b)
    run = _get_exec()
    out_g = run(xt_g, wst_g, bbt_g)
    return _unpack_out(out_g)


# revision 3
# speedup vs baseline: 22.7288x; 1.0357x over previous
"""CrossFeatureFusion TRN2 kernel, v3 — dense-DMA bf16 formulation.

out[i] = x[i] + sum_{j != i} (x[j] @ W[i,j]^T + b[i,j])
x: [4, 65536, 256] f32, W: [4, 4, 256, 256] f32, b: [4, 4, 256] f32.

Data-parallel over N on 8 cores, no collectives.  Per core (NSH=8192 rows):
  - Host packs x-shards feature-major into per-block dense tiles
    xt[nb, k, (j,fc), n] (bf16) so every device DMA is a contiguous
    [128 x 8KB] transfer — no strided descriptors.
  - W blocks pre-transposed host-side into the stationary operand layout
    wst[k, tg, cc, m] (bf16, 1.5MB, loaded once).
  - Per 512-row block: 48 matmuls (12 (i,j) pairs x 2 ec x 2 fc chunks)
    accumulate out^T[i] in 8 PSUM banks; DVE scalar_tensor_tensor drains
    PSUM + bias + residual x[i] -> bf16 out tile; dense DMA out.
  - Host unpacks/upcasts the bf16 out tiles to the final f32 [4,N,256].

Measured (neuron-profile NTFF, For_i repeat differencing): ~173 us/iter
steady state per core, vs a ~166 us PE roofline (768 matmuls x 512 cols
@ 2.4 GHz).  PE runs gap-free and warm; the residual ~4.5 us/iter is the
For_i back-edge barrier + kernel-tail drain.  `prefetch` keeps block 0's
x-tile resident across the back edge (the reload overlaps compute);
`tailopt` reorders the final half's matmul groups and splits its out-DMA
so the back-edge PE gap stays near the HAM re-throttle window.  Full
unrolling (no back edge) is NOT faster: a sustained 100%-duty matmul
stream trips the P0 power downclock (2.4 -> ~2.0 GHz) and loses ~20%.
"""

import sys

if "/opt/trn_rl_repo" not in sys.path:
    sys.path.insert(0, "/opt/trn_rl_repo")

import numpy as np
import ml_dtypes

BF16 = ml_dtypes.bfloat16

M, N, D = 4, 65536, 256
N_CORES = 8
NSH = N // N_CORES  # 8192 rows per core
NB = 512  # rows per block
NBLK = NSH // NB  # 16 blocks per core

_CACHE = {}

INPUT_NAMES = ("xt", "wst", "bbt")


def _build_nc(
    nsh=NSH,
    repeat=1,
    xbufs=4,
    obufs=4,
    pbufs=2,
    out_bf16=True,
    stagger=False,
    hints=False,
    unroll=1,
    prefetch=True,
    tailopt=True,
):
    from concourse import bacc
    import concourse.mybir as mybir
    import concourse.tile as tile

    f32 = mybir.dt.float32
    bf16 = mybir.dt.bfloat16
    nblk = nsh // NB
    add = mybir.AluOpType.add
    odt = bf16 if out_bf16 else f32

    nc = bacc.Bacc(debug=False)
    xt_d = nc.dram_tensor("xt", [nblk, 128, 8 * NB], bf16, kind="ExternalInput")
    wst_d = nc.dram_tensor("wst", [128, 8, 6, 128], bf16, kind="ExternalInput")
    bbt_d = nc.dram_tensor("bbt", [128, 8], f32, kind="ExternalInput")
    out_d = nc.dram_tensor("out", [nblk * 2, 128, 4 * NB], odt, kind="ExternalOutput")

    jl = [[j for j in range(M) if j != i] for i in range(M)]

    with tile.TileContext(nc) as tc:
        with (
            tc.tile_pool(name="wsb", bufs=1) as wpool,
            tc.tile_pool(name="xt", bufs=xbufs) as xpool,
            tc.tile_pool(name="osb", bufs=obufs) as opool,
            tc.tile_pool(name="psum", bufs=pbufs, space="PSUM") as ppool,
        ):
            w_sb = wpool.tile([128, 8, 6, 128], bf16)
            nc.sync.dma_start(out=w_sb[:], in_=wst_d[:])
            bias_sb = wpool.tile([128, 8], f32)
            nc.sync.dma_start(out=bias_sb[:], in_=bbt_d[:])

            # Block 0's x-tile lives in a persistent slot, loaded in the
            # prologue and re-loaded mid-iteration for the next loop trip,
            # so the first matmuls after the back-edge never wait on DMA.
            if prefetch:
                xt0_sb = wpool.tile([128, 8 * NB], bf16)
                nc.sync.dma_start(out=xt0_sb[:], in_=xt_d[0])

            def body():
                for nb in range(nblk):
                    if prefetch and nb == 0:
                        xt_sb = xt0_sb
                    else:
                        xt_sb = xpool.tile(
                            [128, 8 * NB], bf16, name="xt_sb", tag="xt"
                        )
                        nc.sync.dma_start(out=xt_sb[:], in_=xt_d[nb])
                    for half in range(2):
                        # In the loop's final half, run the tt groups in
                        # reverse and split the out-DMA so the kernel tail
                        # (STT + store after the last matmul) is short
                        # enough that the back-edge gap stays under the
                        # HAM re-throttle window.
                        last = tailopt and nb == nblk - 1 and half == 1
                        tts = (3, 2, 1, 0) if last else (0, 1, 2, 3)
                        pss = [
                            ppool.tile(
                                [128, NB], f32, tag=f"ps{t}", name=f"ps{t}_{nb}"
                            )
                            for t in range(4)
                        ]
                        for tt in tts:
                            tg = half * 4 + tt
                            i = tg >> 1
                            for cc in range(6):
                                fc = cc & 1
                                j = jl[i][cc >> 1]
                                c8 = j * 2 + fc
                                nc.tensor.matmul(
                                    pss[tt][:],
                                    lhsT=w_sb[:, tg, cc, :],
                                    rhs=xt_sb[:, c8 * NB : (c8 + 1) * NB],
                                    start=(cc == 0),
                                    stop=(cc == 5),
                                )
                        o_sb = opool.tile(
                            [128, 4 * NB], odt, name=f"osb_{nb}_{half}", tag="osb"
                        )
                        for tt in tts:
                            tg = half * 4 + tt
                            i, ec = tg >> 1, tg & 1
                            r8 = i * 2 + ec
                            nc.vector.scalar_tensor_tensor(
                                out=o_sb[:, tt * NB : (tt + 1) * NB],
                                in0=pss[tt][:],
                                scalar=bias_sb[:, tg : tg + 1],
                                in1=xt_sb[:, r8 * NB : (r8 + 1) * NB],
                                op0=add,
                                op1=add,
                            )
                            if last:
                                nc.sync.dma_start(
                                    out=out_d[2 * nb + half][
                                        :, tt * NB : (tt + 1) * NB
                                    ],
                                    in_=o_sb[:, tt * NB : (tt + 1) * NB],
                                )
                        if not last:
                            nc.sync.dma_start(
                                out=out_d[2 * nb + half], in_=o_sb[:]
                            )
                if prefetch:
                    nc.sync.dma_start(out=xt0_sb[:], in_=xt_d[0])

            if repeat <= unroll:
                for _ in range(repeat):
                    body()
            else:
                assert repeat % unroll == 0
                kw = {}
                if stagger:
                    kw["staggered_reset"] = True
                if hints:
                    kw["hint_engines"] = (
                        mybir.EngineType.PE,
                        mybir.EngineType.DVE,
                        mybir.EngineType.SP,
                    )
                with tc.For_i(0, repeat // unroll, 1, **kw):
                    for _ in range(unroll):
                        body()
    nc.finalize()
    return nc


def _prep_inputs(x, W, b):
    """Host-side pack. Returns global concatenated (xt_g, wst_g, bbt_g)."""
    x = np.asarray(x, dtype=np.float32)
    W = np.asarray(W, dtype=np.float32)
    b = np.asarray(b, dtype=np.float32)

    # xt_g[(c*NBLK + nb), k, (j*2+fc)*NB + n] = x[j, c*NSH + nb*NB + n, fc*128 + k]
    x6 = x.reshape(M, N_CORES, NBLK, NB, 2, 128)  # j c nb n fc k
    xt_g = np.ascontiguousarray(
        x6.transpose(1, 2, 5, 0, 4, 3)
    ).astype(BF16).reshape(N_CORES * NBLK, 128, 8 * NB)

    # wst[k, tg=(i,ec), cc=(jj,fc), m] = W[i, j][ec*128+m, fc*128+k]
    wst = np.empty((128, 8, 6, 128), dtype=np.float32)
    for i in range(M):
        jli = [j for j in range(M) if j != i]
        for ec in range(2):
            t = i * 2 + ec
            for cc in range(6):
                j = jli[cc >> 1]
                fc = cc & 1
                blk = W[i, j][ec * 128 : (ec + 1) * 128, fc * 128 : (fc + 1) * 128]
                wst[:, t, cc, :] = blk.T
    wst_g = np.ascontiguousarray(
        np.broadcast_to(wst.astype(BF16)[None], (N_CORES, 128, 8, 6, 128))
    ).reshape(N_CORES * 128, 8, 6, 128)

    # bias sums: BS[i] = sum_{j != i} b[i, j]; bbt[k, i*2+ec] = BS[i, ec*128+k]
    bs = b.sum(axis=1) - b[np.arange(M), np.arange(M)]  # [4, 256]
    bbt = np.ascontiguousarray(bs.reshape(M, 2, 128).transpose(2, 0, 1)).reshape(
        128, 8
    )
    bbt_g = np.ascontiguousarray(
        np.broadcast_to(bbt[None], (N_CORES, 128, 8))
    ).reshape(N_CORES * 128, 8)

    return xt_g, wst_g, bbt_g


def _unpack_out(out_g):
    """out_g: [NC*NBLK*2, 128, 4*NB] (bf16) -> out [M, N, D] f32."""
    v = np.asarray(out_g).reshape(N_CORES, NBLK, 2, 128, 2, 2, NB)
    # dims: c nb half m tthi ec n ; i = half*2 + tthi, e = ec*128 + m
    out = v.transpose(2, 4, 0, 1, 6, 5, 3).reshape(M, N, D)
    return np.ascontiguousarray(out).astype(np.float32)


def _get_exec(**build_kwargs):
    """Build (once per config) the jitted 8-core executor."""
    key = tuple(sorted(build_kwargs.items()))
    if key in _CACHE:
        return _CACHE[key]

    import jax
    import jax.numpy as jnp
    from jax.sharding import Mesh, PartitionSpec
    from jax.experimental.shard_map import shard_map
    from concourse import bass2jax

    nc = _build_nc(**build_kwargs)
    bass2jax.install_neuronx_cc_hook()

    out_bf16 = build_kwargs.get("out_bf16", True)
    odt = jnp.bfloat16 if out_bf16 else np.float32

    in_names = list(INPUT_NAMES) + ["out"]
    if nc.partition_id_tensor is not None:
        in_names.append(nc.partition_id_tensor.name)
    out_names = ["out"]
    out_aval = jax.core.ShapedArray((NBLK * 2, 128, 4 * NB), odt)

    def _body(xt, wst, bbt, out_zero):
        operands = [xt, wst, bbt, out_zero]
        if nc.partition_id_tensor is not None:
            operands.append(bass2jax.partition_id_tensor())
        outs = bass2jax._bass_exec_p.bind(
            *operands,
            out_avals=(out_aval,),
            in_names=tuple(in_names),
            out_names=tuple(out_names),
            lowering_input_output_aliases=(),
            sim_require_finite=True,
            sim_require_nnan=True,
            nc=nc,
        )
        return tuple(outs)

    devices = jax.devices()[:N_CORES]
    mesh = Mesh(np.asarray(devices), ("core",))
    sharded = jax.jit(
        shard_map(
            _body,
            mesh=mesh,
            in_specs=(PartitionSpec("core"),) * 4,
            out_specs=(PartitionSpec("core"),),
            check_rep=False,
        ),
        donate_argnums=(3,),
        keep_unused=True,
    )

    sharding = jax.sharding.NamedSharding(mesh, PartitionSpec("core"))
    zeros_fn = jax.jit(
        lambda: jnp.zeros((N_CORES * NBLK * 2, 128, 4 * NB), odt),
        out_shardings=sharding,
    )

    def run(xt_g, wst_g, bbt_g, n_iters=1):
        xt_j = jax.device_put(xt_g, sharding)
        wst_j = jax.device_put(wst_g, sharding)
        bbt_j = jax.device_put(bbt_g, sharding)
        outs = None
        for _ in range(n_iters):
            outs = sharded(xt_j, wst_j, bbt_j, zeros_fn())
        jax.block_until_ready(outs)
        return outs[0]

    _CACHE[key] = run
    return run


def kernel(x, W, b):
    xt_g, wst_g, bbt_g = _prep_inputs(x, W, b)
    run = _get_exec()
    out_g = run(xt_g, wst_g, bbt_g)
    return _unpack_out(out_g)
